# revision 18
# baseline (speedup 1.0000x reference)
"""Trainium2 Bass kernel for BlockUncertaintyTracker (segment_reduce).

Computes, per 4x4 block of a [16,1,2048,2048] image batch:
  - mean over the 16 block elements, averaged over batch
  - 0.9-quantile (= 0.5*(2nd largest + 3rd largest)), averaged over batch
  - EMA update of both stats, then broadcasts the ratio back to full shape.

Sharding: spatial over H across 8 cores (256 image rows / 64 block rows per
core). Every core sees all 16 batch elements for its rows, so no collectives
are needed; EMA buffer slices are contiguous per core.

Pipeline: 4 column chunks (512 cols each) x 2 supergroups (8 batches each).
Within a (chunk, supergroup) unit the 4 groups' data is CONCATENATED along
the free dim so every DVE merge op runs at the full 2048/1024/512 widths
(amortizing fixed per-op cost) while chunking lets each chunk's 8 MiB output
write overlap the next chunk's compute.

Engines: Act casts f32->f16 and does most even/odd deinterleaves; Pool
(gpsimd) takes a share of deints; DVE runs the 26-op sorted-3 merge network
in f16 2x mode; PE accumulates block sums AND the quantile stat with f16
matmuls (ones lhsT folds batch pairs + duplicates rows for the output
layout); per-chunk tail computes the EMA ratio and one broadcast-source DMA
replicates it to all batches.
"""

import os

import numpy as np

# ---- problem constants (hardcoded; kernel.py must be self-contained) ----
B = 16          # batch
H = 2048
W = 2048
BS = 4          # block size
NCORES = 8
HS = H // NCORES            # 256 rows per core
NBH = HS // BS              # 64 block rows per core
NBW = W // BS               # 512 block cols
ROWS = B * HS               # 4096 rows in a per-core slab
CHUNKS = (512, 512, 512, 320, 192)   # uneven column chunks (sum = W);
                                     # small final chunk shrinks the exposed
                                     # end-of-kernel output write
NSG = 2                     # supergroups (8 batches each)
GPS = 4                     # groups processed side by side (2 batches each)
DECAY = 0.99
ALPHA = 0.1
EPS = 1e-5
C_MEAN = (1.0 - DECAY) / (BS * BS * B)    # fold mean-over-16-elems and batch
C_QUANT = (1.0 - DECAY) * 0.5 / B         # fold 0.5*(m2+m3) and batch mean

_CACHE = {}


def _split_multi_waits(nc):
    """This walrus build encodes at most ONE sync wait per instruction.
    Tile attaches several. Hoist excess waits onto same-engine NOPs placed
    immediately before the owning instruction (same engine stream => same
    semantics)."""
    import concourse.mybir as mybir

    plans = []  # (inst_name, extra_waits)
    for f in nc.m.functions:
        for bb in f.blocks:
            for inst in bb.instructions:
                si = getattr(inst, "sync_info", None)
                waits = list(si.on_wait) if (si and si.on_wait) else []
                if len(waits) > 1:
                    si.on_wait = [waits[-1]]
                    plans.append((inst.name, waits[:-1]))

    if not plans:
        return

    nop_for = {}
    stray = set()
    for iname, extra in plans:
        nops = []
        for w in extra:
            nop = nc.engines[nc.inst_map[iname].engine].nop(nofuse=True).ins
            nop.sync_info = mybir.SyncInfo(on_wait=[w], on_update=[])
            nops.append(nop)
            stray.add(nop.name)
        nop_for[iname] = nops

    for f in nc.m.functions:
        for bb in f.blocks:
            out = []
            changed = False
            for inst in bb.instructions:
                if inst.name in stray:
                    changed = True
                    continue
                if inst.name in nop_for:
                    out.extend(nop_for[inst.name])
                    changed = True
                out.append(inst)
            if changed:
                bb.instructions = out


def _build():
    """Builds the single-core Bass program (SPMD across 8 cores)."""
    from contextlib import ExitStack

    import concourse.bass as bass
    import concourse.mybir as mybir
    import concourse.tile as tile

    f32 = mybir.dt.float32
    f16 = mybir.dt.float16
    MAX = mybir.AluOpType.max
    MIN = mybir.AluOpType.min
    MULT = mybir.AluOpType.mult
    ADD = mybir.AluOpType.add

    nc = bass.Bass("TRN2", target_bir_lowering=False, debug=False)

    x = nc.dram_tensor("x", [ROWS, W], f32, kind="ExternalInput").ap()
    ee = nc.dram_tensor("ee", [NBH, NBW], f32, kind="ExternalInput").ap()
    eq = nc.dram_tensor("eq", [NBH, NBW], f32, kind="ExternalInput").ap()
    # ones16[p, m] = (p % 64 == m // 2): batch-pair fold + row duplication
    ones16 = nc.dram_tensor("ones16", [128, 128], f16, kind="ExternalInput").ap()
    y = nc.dram_tensor("y", [ROWS, W], f32, kind="ExternalOutput").ap()



    with tile.TileContext(nc) as tc, ExitStack() as ctx:
        pool = ctx.enter_context(tc.tile_pool(name="work", bufs=1))
        ppool = ctx.enter_context(tc.tile_pool(name="acc", bufs=1, space="PSUM"))

        ones_sb = pool.tile([128, 128], f16, tag="ones")

        def tt(dst, a, bb, op):
            nc.vector.tensor_tensor(dst, a, bb, op)

        NMM = 16  # sum matmuls per (ch, sg) unit

        # unit list: (uid, ch, sg, c0, CW); software-pipelined emission
        units = []
        offs = [0]
        for cwc in CHUNKS:
            offs.append(offs[-1] + cwc)
        for ch, cwc in enumerate(CHUNKS):
            for sg in range(NSG):
                units.append((ch, sg, offs[ch], cwc))
        NU = len(units)

        st = {}   # uid -> unit tiles
        cst = {}  # ch -> chunk tiles (psum, ee)

        def emit_load(uid):
            ch, sg, c0, CW = units[uid]
            u = f"{ch}_{sg}"
            xr = x[:, c0 : c0 + CW].rearrange(
                "(sg gg b2 i r) w -> sg r (b2 i) gg w",
                sg=NSG, gg=GPS, b2=2, i=NBH, r=BS,
            )
            rts = []
            for r in range(BS):
                rt = pool.tile(
                    [128, GPS * CW], f32, tag=f"in{r}", bufs=2, name=f"rt{r}_{u}"
                )
                rtv = rt.rearrange("p (g w) -> p g w", g=GPS)
                if uid < 2:
                    # startup ramp: half-partition DMAs so the first row
                    # phases land (and cast) before the rest finish
                    nc.sync.dma_start(rtv[0:64], xr[sg, r][0:64])
                    nc.sync.dma_start(rtv[64:128], xr[sg, r][64:128])
                else:
                    nc.sync.dma_start(rtv, xr[sg, r])
                rts.append(rt)
            st[uid] = {"rts": rts}
            if sg == 0:
                CB = CW // BS
                ee_sb = pool.tile([128, CB], f32, tag="eesb", bufs=2,
                                  name=f"ee{ch}")
                nc.gpsimd.dma_start(
                    ee_sb[:, :],
                    ee[:, c0 // BS : c0 // BS + CB]
                    .unsqueeze(1).broadcast_to((NBH, 2, CB)),
                )
                eq_sb = pool.tile([128, CB], f32, tag="eqsb", bufs=2,
                                  name=f"eq{ch}")
                nc.gpsimd.dma_start(
                    eq_sb[:, :],
                    eq[:, c0 // BS : c0 // BS + CB]
                    .unsqueeze(1).broadcast_to((NBH, 2, CB)),
                )
                cst[ch] = {"ee_sb": ee_sb, "eq_sb": eq_sb}

        def emit_cast(uid):
            ch, sg, c0, CW = units[uid]
            u = f"{ch}_{sg}"
            bts = []
            for r in range(BS):
                bt = pool.tile(
                    [128, GPS * CW], f16, tag=f"bc{r}", bufs=2, name=f"bt{r}_{u}"
                )
                nc.scalar.copy(bt[:, :], st[uid]["rts"][r][:, :])
                bts.append(bt)
            st[uid]["bts"] = bts
            if sg == 0:
                CB = CW // BS
                c = cst[ch]
                ee2 = pool.tile([128, CB], f32, tag="ee2", bufs=2,
                                name=f"ee2_{ch}")
                nc.scalar.activation(
                    ee2[:, :], c["ee_sb"][:, :],
                    mybir.ActivationFunctionType.Copy, bias=EPS, scale=DECAY,
                )
                eq2 = pool.tile([128, CB], f32, tag="eq2", bufs=2,
                                name=f"eq2_{ch}")
                nc.scalar.activation(
                    eq2[:, :], c["eq_sb"][:, :],
                    mybir.ActivationFunctionType.Copy, bias=0.0, scale=DECAY,
                )
                c["ee2"], c["eq2"] = ee2, eq2

        def emit_mergeV(uid):
            ch, sg, c0, CW = units[uid]
            u = f"{ch}_{sg}"
            GW = GPS * CW
            b0, b1, b2_, b3 = st[uid]["bts"]

            if sg == 0:
                cst[ch]["ps"] = ppool.tile([128, CW], f32, tag="ps", bufs=2,
                                           name=f"ps{ch}")
                cst[ch]["pq"] = ppool.tile([128, CW], f32, tag="pq", bufs=2,
                                           name=f"pq{ch}")
            psum_s = cst[ch]["ps"]

            def vt(name, tag, bufs=1):
                return pool.tile([128, GW], f16, tag=tag, bufs=bufs,
                                 name=f"{name}_{u}")

            v1 = vt("v1", "v1")
            tt(v1[:, :], b0[:, :], b1[:, :], MAX)
            w1v = vt("w1v", "w1v")
            tt(w1v[:, :], b0[:, :], b1[:, :], MIN)
            v2 = vt("v2", "v2")
            tt(v2[:, :], b2_[:, :], b3[:, :], MAX)
            w2v = vt("w2v", "w2v")
            tt(w2v[:, :], b2_[:, :], b3[:, :], MIN)
            m = vt("m", "m", 2)
            tt(m[:, :], v1[:, :], v2[:, :], MAX)
            t1 = vt("t1", "t1")
            tt(t1[:, :], v1[:, :], v2[:, :], MIN)
            t2 = vt("t2", "t2")
            tt(t2[:, :], w1v[:, :], w2v[:, :], MAX)
            s2 = vt("s2", "s2", 2)
            tt(s2[:, :], t1[:, :], t2[:, :], MAX)
            t3 = vt("t3", "t3", 2)
            tt(t3[:, :], t1[:, :], t2[:, :], MIN)
            st[uid].update(m=m, s2=s2, t3=t3)

            # sum path: v1+w1v+v2+w2v == b0+b1+b2+b3 elementwise (pair
            # min/max preserves sums): exact f16 block column-sums with
            # batch-pair fold + row duplication.
            k0 = sg * NMM
            for ti, vtile in enumerate((v1, w1v, v2, w2v)):
                for g in range(GPS):
                    k = k0 + ti * GPS + g
                    nc.tensor.matmul(
                        psum_s[:, :], lhsT=ones_sb[:, :],
                        rhs=vtile[:, g * CW : (g + 1) * CW],
                        start=(k == 0), stop=(k == NSG * NMM - 1),
                    )

        def emit_mergeAB(uid):
            ch, sg, c0, CW = units[uid]
            u = f"{ch}_{sg}"
            GW = GPS * CW
            HW2 = GW // 2
            QW = HW2 // 2
            m, s2, t3 = st[uid]["m"], st[uid]["s2"], st[uid]["t3"]
            psum_q = cst[ch]["pq"]

            # A level: deint layout fuses cross ops: mEt=[me|s2e], mOt=[s2o|mo]
            def deint(src, dst_ap, par):
                v = src.rearrange("p (g j two) -> p g j two", g=GPS, two=2)
                nc.scalar.copy(
                    dst_ap.rearrange("p (g j) -> p g j", g=GPS),
                    v[:, :, :, par],
                )

            mEt = pool.tile([128, GW], f16, tag="mEt", bufs=1, name=f"mEt_{u}")
            mOt = pool.tile([128, GW], f16, tag="mOt", bufs=1, name=f"mOt_{u}")
            deint(m, mEt[:, 0:HW2], 0)        # me
            deint(m, mOt[:, HW2:GW], 1)       # mo
            deint(s2, mEt[:, HW2:GW], 0)      # s2e
            deint(s2, mOt[:, 0:HW2], 1)       # s2o
            t3e = pool.tile([128, HW2], f16, tag="t3e", bufs=1, name=f"t3e_{u}")
            deint(t3, t3e[:, :], 0)
            t3o = pool.tile([128, HW2], f16, tag="t3o", bufs=1, name=f"t3o_{u}")
            deint(t3, t3o[:, :], 1)

            me, mo = mEt[:, 0:HW2], mOt[:, HW2:GW]
            s2e, s2o = mEt[:, HW2:GW], mOt[:, 0:HW2]

            def mid(name, wd=HW2):
                return pool.tile([128, wd], f16, tag=name, bufs=1,
                                 name=f"{name}_{u}")

            p1 = mid("p1")
            tt(p1[:, :], me, mo, MAX)
            u1 = mid("u1")
            tt(u1[:, :], me, mo, MIN)
            u2 = mid("u2")
            tt(u2[:, :], s2e, s2o, MAX)
            p2 = mid("p2")
            tt(p2[:, :], u1[:, :], u2[:, :], MAX)
            # w23 = [min(me,s2o) | min(s2e,mo)] in one fused op
            w23 = mid("w23", GW)
            tt(w23[:, :], mEt[:, :], mOt[:, :], MIN)
            w4 = mid("w4")
            tt(w4[:, :], w23[:, 0:HW2], w23[:, HW2:GW], MAX)
            w1 = mid("w1")
            tt(w1[:, :], t3e[:, :], t3o[:, :], MAX)
            p3 = mid("p3")
            tt(p3[:, :], w1[:, :], w4[:, :], MAX)

            # B level: pzE = [p1e|p2e], pzO = [p2o|p1o] (swapped) fuses z4/z5
            pzE = pool.tile([128, HW2], f16, tag="pzE", bufs=1, name=f"pzE_{u}")
            pzO = pool.tile([128, HW2], f16, tag="pzO", bufs=1, name=f"pzO_{u}")
            deint(p1, pzE[:, 0:QW], 0)        # p1e
            deint(p1, pzO[:, QW:HW2], 1)      # p1o
            deint(p2, pzE[:, QW:HW2], 0)      # p2e
            deint(p2, pzO[:, 0:QW], 1)        # p2o
            p3e = pool.tile([128, QW], f16, tag="p3e", bufs=1, name=f"p3e_{u}")
            deint(p3, p3e[:, :], 0)
            p3o = pool.tile([128, QW], f16, tag="p3o", bufs=1, name=f"p3o_{u}")
            deint(p3, p3o[:, :], 1)

            p1e, p1o = pzE[:, 0:QW], pzO[:, QW:HW2]
            p2e, p2o = pzE[:, QW:HW2], pzO[:, 0:QW]

            def small(name, wd=QW):
                return pool.tile([128, wd], f16, tag=name, bufs=1,
                                 name=f"{name}_{u}")

            z1 = small("z1")
            tt(z1[:, :], p1e, p1o, MIN)
            z2 = small("z2")
            tt(z2[:, :], p2e, p2o, MAX)
            c2 = small("c2")
            tt(c2[:, :], z1[:, :], z2[:, :], MAX)
            # z45 = [min(p1e,p2o) | min(p2e,p1o)] in one fused op
            z45 = small("z45", HW2)
            tt(z45[:, :], pzE[:, :], pzO[:, :], MIN)
            z6 = small("z6")
            tt(z6[:, :], z45[:, 0:QW], z45[:, QW:HW2], MAX)
            z3 = small("z3")
            tt(z3[:, :], p3e[:, :], p3o[:, :], MAX)
            c3 = small("c3")
            tt(c3[:, :], z3[:, :], z6[:, :], MAX)

            # quantile stat: psum_q += fold(c2) + fold(c3)
            nc.tensor.matmul(
                psum_q[:, :], lhsT=ones_sb[:, :], rhs=c2[:, :],
                start=(sg == 0), stop=False,
            )
            nc.tensor.matmul(
                psum_q[:, :], lhsT=ones_sb[:, :], rhs=c3[:, :],
                start=False, stop=(sg == NSG - 1),
            )

        def emit_tail(ch):
            c0 = offs[ch]
            CW = CHUNKS[ch]
            CB = CW // BS
            c = cst[ch]
            yr = y[:, c0 : c0 + CW].rearrange(
                "(b i h r2) w -> h r2 i b w", b=B, i=NBH, h=2, r2=2,
            )

            def tail_tile(name, wdt=CB, dt=f32):
                return pool.tile([128, wdt], dt, tag=name, bufs=2,
                                 name=f"{name}_{ch}")

            # S[p, j] = sum over the 4 columns of each block
            S = tail_tile("S")
            nc.vector.tensor_reduce(
                S[:, :], c["ps"].rearrange("p (j cc) -> p j cc", cc=BS),
                mybir.AxisListType.X, ADD,
            )
            # qf[p, j] = sum over the 4 concatenated groups
            qf = tail_tile("qf")
            nc.vector.tensor_reduce(
                qf[:, :], c["pq"].rearrange("p (g j) -> p j g", g=GPS),
                mybir.AxisListType.X, ADD,
            )

            den = tail_tile("den")
            nc.vector.scalar_tensor_tensor(
                den[:, :], S[:, :], C_MEAN, c["ee2"][:, :], op0=MULT, op1=ADD
            )
            num = tail_tile("num")
            nc.vector.scalar_tensor_tensor(
                num[:, :], qf[:, :], C_QUANT, c["eq2"][:, :], op0=MULT, op1=ADD
            )
            rec = tail_tile("rec")
            nc.vector.reciprocal(rec[:, :], den[:, :])
            uu = tail_tile("uu")
            nc.vector.tensor_tensor(uu[:, :], num[:, :], rec[:, :], MULT)

            # expand x4 along columns on Act: u4[p, j*4 + cc] = uu[p, j]
            u4 = tail_tile("u4", CW)
            u4v = u4.rearrange("p (j cc) -> p j cc", cc=BS)
            for cc in range(BS):
                nc.gpsimd.tensor_copy(u4v[:, :, cc], uu[:, :])

            # broadcast-source DMAs per (h, r2): dst/src both [64, B, CW]
            u4r = u4.rearrange("(i r2) w -> r2 i w", r2=2)
            for r2 in range(2):
                u4b = u4r[r2].unsqueeze(1).broadcast_to((NBH, B, CW))
                for h in range(2):
                    nc.gpsimd.dma_start(yr[h, r2], u4b)

        # ---- software-pipelined emission ----
        emit_load(0)
        nc.sync.dma_start(ones_sb[:, :], ones16)
        emit_cast(0)
        emit_load(1)
        for uid in range(NU):
            emit_mergeV(uid)
            if uid + 1 < NU:
                emit_cast(uid + 1)
            emit_mergeAB(uid)
            if uid + 2 < NU:
                emit_load(uid + 2)
            if units[uid][1] == NSG - 1:
                emit_tail(units[uid][0])

    _split_multi_waits(nc)
    return nc



def _get_nc():
    if "nc" not in _CACHE:
        _CACHE["nc"] = _build()
    return _CACHE["nc"]


def kernel(current_errors, ema_errors, ema_quantile):
    from concourse.bass_utils import run_bass_kernel_spmd

    x = np.asarray(current_errors, dtype=np.float32).reshape(B, H, W)
    ee = np.asarray(ema_errors, dtype=np.float32).reshape(H // BS, W // BS)
    eq = np.asarray(ema_quantile, dtype=np.float32).reshape(H // BS, W // BS)

    # ones16[p, m] == 1 iff p % 64 == m // 2 (batch-pair fold + row dup)
    ones16 = np.zeros((128, 128), dtype=np.float16)
    p = np.arange(128)
    ones16[p, (p % NBH) * 2] = 1.0
    ones16[p, (p % NBH) * 2 + 1] = 1.0

    in_maps = []
    for k in range(NCORES):
        xs = np.ascontiguousarray(x[:, k * HS : (k + 1) * HS, :]).reshape(ROWS, W)
        ees = np.ascontiguousarray(ee[k * NBH : (k + 1) * NBH, :])
        eqs = np.ascontiguousarray(eq[k * NBH : (k + 1) * NBH, :])
        in_maps.append({"x": xs, "ee": ees, "eq": eqs, "ones16": ones16})

    nc = _get_nc()
    trace = bool(int(os.environ.get("KERNEL_TRACE", "0")))
    try:
        res = run_bass_kernel_spmd(
            nc, in_maps, core_ids=list(range(NCORES)), trace=trace
        )
    except Exception:
        # transient device state (e.g. NRT_EXEC_UNIT_UNRECOVERABLE) — retry once
        res = run_bass_kernel_spmd(
            nc, in_maps, core_ids=list(range(NCORES)), trace=trace
        )
    _CACHE["last_results"] = res

    out = np.empty((B, 1, H, W), dtype=np.float32)
    for k in range(NCORES):
        out[:, 0, k * HS : (k + 1) * HS, :] = res.results[k]["y"].reshape(B, HS, W)
    return out
